# revision 10
# baseline (speedup 1.0000x reference)
"""Chunked-attention Trainium2 kernel.

Problem (hardcoded shapes): x [4, 16384, 256] f32, in_proj_w [768, 256],
in_proj_b [768], out_w [256, 256], out_b [256].

Reference semantics: overlapping 128-token chunks (stride 96, overlap 32),
fused qkv projection, 8-head attention within each chunk, out projection,
overlap-add with divisor normalization.

Distribution: 8 cores = (batch b in 0..3) x (chunk-half in 0..1). Each core
processes 86 chunks of one batch (halves share chunk 85; the host drops the
duplicate). Each core receives its token window of x pre-transposed/cast to
bf16 ([d, t] layout) and emits raw per-chunk outputs [86, 128, 256] f32.
Overlap-add + divisor + bias constants are applied on the host (linear ops
that commute with the out projection).

Device pipeline per core:
  phase 1 (per 128-token tile): qkv projection for q,k in [e, tok] layout
    (weights stationary, xT moving), psum->sbuf copies with bias add + bf16
    cast.
  phase 2 (per chunk): v projection in [tok, e] layout; S^T = k_h q_h^T per
    head via row-packed tile_position matmuls (K=32); exp via ACT (scale
    folded); softmax denominators via ones-matmul (col-packed, broadcast to
    32 rows); O^T via col-packed matmuls (v_h stationary, P^T moving);
    normalization fused into the psum->sbuf copy; out projection; store.
"""

import numpy as np
import ml_dtypes
from contextlib import ExitStack

import concourse.bass as bass
import concourse.bacc as bacc
import concourse.mybir as mybir
import concourse.tile as tile
from concourse.bass_utils import run_bass_kernel_spmd


def _install_axon_ntff_hook():
    """Provide antenv.axon_hooks if the image lacks it, wired to the
    libaxon_pjrt.so NTFF profile ABI, so trace=True works under axon."""
    import sys, types, contextlib, ctypes
    try:
        from antenv.axon_hooks import get_axon_ntff_profile_hook  # noqa: F401
        return
    except ImportError:
        pass
    mod = types.ModuleType("antenv.axon_hooks")
    holder = [None]
    mod.set_axon_ntff_profile_hook = lambda h: holder.__setitem__(0, h)
    mod.get_axon_ntff_profile_hook = lambda: holder[0]
    sys.modules["antenv.axon_hooks"] = mod
    try:
        import antenv
        antenv.axon_hooks = mod
    except ImportError:
        pass
    so_path = "/opt/axon/libaxon_pjrt.so"
    try:
        lib = ctypes.CDLL(so_path)
        if not hasattr(lib, "axon_start_nrt_profile"):
            return
        lib.axon_start_nrt_profile.argtypes = [
            ctypes.POINTER(ctypes.c_int64), ctypes.c_size_t]
        lib.axon_start_nrt_profile.restype = ctypes.c_int64
        lib.axon_stop_nrt_profile.argtypes = [ctypes.c_char_p]
        lib.axon_stop_nrt_profile.restype = ctypes.c_int64

        @contextlib.contextmanager
        def _hook(output_dir, device_ids):
            import jax
            jax.devices()
            if device_ids:
                ids = (ctypes.c_int64 * len(device_ids))(*device_ids)
                rc = lib.axon_start_nrt_profile(ids, len(device_ids))
            else:
                rc = lib.axon_start_nrt_profile(None, 0)
            if rc != 0:
                raise RuntimeError(f"axon_start_nrt_profile rc={rc}")
            try:
                yield
            finally:
                n = lib.axon_stop_nrt_profile(str(output_dir).encode())
                print(f"profile: {n} file(s) written to {output_dir}")

        holder[0] = _hook
    except OSError:
        pass


_install_axon_ntff_hook()

F32 = mybir.dt.float32
BF16 = mybir.dt.bfloat16

DIM = 256
CHUNK = 128
OVERLAP = 32
STRIDE = 96
HEADS = 8
HD = 32
B = 4
T = 16384
L = 171              # total chunks per batch
T_PAD = L * STRIDE + OVERLAP  # 16448
SCALE = float(1.0 / np.sqrt(HD))

N_CHUNKS = 86        # chunks per core
N_TILES = 65         # 128-token qkv tiles per core
NTOK = N_TILES * 128  # 8320 padded local tokens (needs 86*96+32 = 8288)
T_HOST_PAD = STRIDE * (N_CHUNKS - 1) + NTOK  # 8160 + 8320 = 16480
SEG_TILES = 13       # xt DMA segment granularity (13 tiles = 1664 tokens)

_BASS_CACHE = {}


def build_bass(n_chunks=N_CHUNKS, n_tiles=N_TILES):
    key = (n_chunks, n_tiles)
    if key in _BASS_CACHE:
        return _BASS_CACHE[key]
    ntok = n_tiles * 128
    nc = bacc.Bacc(trn_type="TRN2", target_bir_lowering=False, debug=False)
    xt_d = nc.dram_tensor("xt", [2, 128, ntok], BF16, kind="ExternalInput")
    wt_d = nc.dram_tensor("wt", [2, 128, 768], BF16, kind="ExternalInput")
    wto_d = nc.dram_tensor("wto", [2, 128, 256], BF16, kind="ExternalInput")
    bqk_d = nc.dram_tensor("bqk", [128, 4], F32, kind="ExternalInput")
    y_d = nc.dram_tensor("y", [n_chunks, 128, 256], F32, kind="ExternalOutput")

    with tile.TileContext(nc) as tc, ExitStack() as ctx:
        _body(ctx, tc, xt_d[:], wt_d[:], wto_d[:], bqk_d[:], y_d[:],
              n_chunks, n_tiles)
    nc.compile()
    _BASS_CACHE[key] = nc
    return nc


def _body(ctx, tc, xt_d, wt_d, wto_d, bqk_d, y_d, n_chunks, n_tiles):
    nc = tc.nc
    ntok = n_tiles * 128

    consts = ctx.enter_context(tc.tile_pool(name="consts", bufs=1))
    big = ctx.enter_context(tc.tile_pool(name="big", bufs=1))
    sb = ctx.enter_context(tc.tile_pool(name="sb", bufs=3))
    sb2 = ctx.enter_context(tc.tile_pool(name="sb2", bufs=2))
    ps_qk = ctx.enter_context(tc.tile_pool(name="ps_qk", bufs=1, space="PSUM"))
    ps_st = ctx.enter_context(tc.tile_pool(name="ps_st", bufs=1, space="PSUM"))
    ps_so = ctx.enter_context(tc.tile_pool(name="ps_so", bufs=1, space="PSUM"))
    ps_vy = ctx.enter_context(tc.tile_pool(name="ps_vy", bufs=2, space="PSUM"))

    # constants
    wt_sb = consts.tile([128, 2, 768], BF16)
    for j in range(2):
        nc.sync.dma_start(wt_sb[:, j, :], wt_d[j])
    wto_sb = consts.tile([128, 2, 256], BF16)
    for g in range(2):
        nc.sync.dma_start(wto_sb[:, g, :], wto_d[g])
    bqk_sb = consts.tile([128, 4], F32)
    nc.sync.dma_start(bqk_sb[:], bqk_d[:])
    ones_sb = consts.tile([128, 32], BF16)
    nc.vector.memset(ones_sb[:], 1.0)

    # span buffers
    xt_sb = big.tile([128, 2, ntok], BF16)
    qT_sb = big.tile([128, 2, ntok], BF16)
    kT_sb = big.tile([128, 2, ntok], BF16)

    n_segs = (n_tiles + SEG_TILES - 1) // SEG_TILES

    def load_seg(s):
        a = s * SEG_TILES * 128
        b = min(ntok, (s + 1) * SEG_TILES * 128)
        for j in range(2):
            nc.sync.dma_start(xt_sb[:, j, a:b], xt_d[j][:, a:b])

    def qkv_tile(t):
        a = t * 128
        qk_ps = ps_qk.tile([128, 4, 128], F32, tag="qk")
        for s in range(4):
            for j in range(2):
                nc.tensor.matmul(
                    qk_ps[:, s, :],
                    wt_sb[:, j, 128 * s:128 * (s + 1)],
                    xt_sb[:, j, a:a + 128],
                    start=(j == 0), stop=(j == 1),
                )
        # q slices on ACT (bias add + bf16 cast), k slices on DVE
        for s in range(2):
            nc.scalar.activation(
                qT_sb[:, s, a:a + 128], qk_ps[:, s, :],
                mybir.ActivationFunctionType.Identity,
                bias=bqk_sb[:, s:s + 1], scale=1.0,
            )
        for s in range(2, 4):
            nc.vector.tensor_scalar_add(
                kT_sb[:, s - 2, a:a + 128], qk_ps[:, s, :],
                bqk_sb[:, s:s + 1],
            )

    def attn_chunk(c):
        tok0 = c * STRIDE
        # v projection: [tok, e] layout, xT stationary
        v_ps = ps_vy.tile([128, 256], F32, tag="vy")
        for j in range(2):
            nc.tensor.matmul(
                v_ps[:], xt_sb[:, j, tok0:tok0 + 128], wt_sb[:, j, 512:768],
                start=(j == 0), stop=(j == 1),
            )
        v_sb = sb.tile([128, 256], BF16, tag="v")
        nc.vector.tensor_copy(v_sb[:], v_ps[:])

        so_ps = ps_so.tile([128, 2, 256], F32, tag="so")  # [ot | sums]
        oc_sb = sb2.tile([128, 2, 128], BF16, tag="oc")
        for g in range(2):
            # concurrent row-strip matmuls must land in distinct PSUM banks:
            # [128, 4, 512] spans 4 banks, head h writes bank h cols 0:128
            st_ps = ps_st.tile([128, 4, 512], F32, tag="st")
            for h in range(4):
                p0 = 32 * h
                nc.tensor.matmul(
                    st_ps[:, h, 0:128],
                    kT_sb[p0:p0 + 32, g, tok0:tok0 + 128],
                    qT_sb[p0:p0 + 32, g, tok0:tok0 + 128],
                    start=True, stop=True,
                    tile_position=(p0, 0),
                )
            pt_sb = sb.tile([128, 4, 128], BF16, tag="pt")
            nc.scalar.activation(
                pt_sb[:], st_ps[:, :, 0:128],
                mybir.ActivationFunctionType.Exp, scale=SCALE,
            )
            for h in range(4):
                p0 = 32 * h
                nc.tensor.matmul(
                    so_ps[p0:p0 + 32, g, 128:256],
                    ones_sb[:, :32], pt_sb[:, h, :],
                    start=True, stop=True,
                    tile_position=(0, p0),
                )
                nc.tensor.matmul(
                    so_ps[p0:p0 + 32, g, 0:128],
                    v_sb[:, 32 * (4 * g + h):32 * (4 * g + h) + 32],
                    pt_sb[:, h, :],
                    start=True, stop=True,
                    tile_position=(0, p0),
                )
            rb_sb = sb.tile([128, 128], F32, tag="rb")
            nc.vector.reciprocal(rb_sb[:], so_ps[:, g, 128:256])
            nc.vector.tensor_mul(oc_sb[:, g, :], so_ps[:, g, 0:128], rb_sb[:])

        y_ps = ps_vy.tile([128, 256], F32, tag="vy")
        for g in range(2):
            nc.tensor.matmul(
                y_ps[:], oc_sb[:, g, :], wto_sb[:, g, :],
                start=(g == 0), stop=(g == 1),
            )
        y_sb = sb.tile([128, 256], F32, tag="y")
        nc.scalar.activation(y_sb[:, 0:128], y_ps[:, 0:128],
                             mybir.ActivationFunctionType.Copy)
        nc.vector.tensor_copy(y_sb[:, 128:256], y_ps[:, 128:256])
        nc.sync.dma_start(y_d[c], y_sb[:])

    # interleaved emission: load xt segments / project qkv tiles lazily,
    # then attention chunks as their token windows become available
    next_seg = 0
    next_tile = 0
    for c in range(n_chunks):
        t_hi = (c * STRIDE + 128 - 1) // 128  # last tile index chunk c needs
        while next_tile <= t_hi:
            while next_seg * SEG_TILES <= next_tile and next_seg < n_segs:
                load_seg(next_seg)
                next_seg += 1
            qkv_tile(next_tile)
            next_tile += 1
        attn_chunk(c)


def _shard_inputs(x, in_proj_w, in_proj_b):
    """Build per-core input maps. Core i = (batch i//2, half i%2)."""
    xp = np.zeros((B, T_HOST_PAD, DIM), np.float32)
    xp[:, :T, :] = x
    wt = np.ascontiguousarray(in_proj_w.T.reshape(2, 128, 768)).astype(
        ml_dtypes.bfloat16)
    bqk = np.ascontiguousarray(in_proj_b[:512].reshape(4, 128).T).astype(
        np.float32)
    in_maps = []
    for core in range(8):
        b, half = core // 2, core % 2
        t0 = half * STRIDE * (N_CHUNKS - 1)  # 0 or 8160
        seg = xp[b, t0:t0 + NTOK, :]
        xt = np.ascontiguousarray(seg.T.reshape(2, 128, NTOK)).astype(
            ml_dtypes.bfloat16)
        in_maps.append({"xt": xt, "wt": wt, "wto": None, "bqk": bqk})
    return in_maps


def _assemble(y_cores, in_proj_w, in_proj_b, out_w, out_b, dtype):
    """Host overlap-add + divisor + bias constants."""
    out = np.zeros((B, T_PAD, DIM), np.float32)
    for b in range(B):
        ych = np.empty((L, 128, DIM), np.float32)
        ych[:N_CHUNKS] = y_cores[2 * b]
        ych[N_CHUNKS:] = y_cores[2 * b + 1][1:]  # drop duplicated chunk 85
        ov = out[b, :L * STRIDE].reshape(L, STRIDE, DIM)
        ov += ych[:, :STRIDE]
        ov[1:, :OVERLAP] += ych[:-1, STRIDE:]
        out[b, L * STRIDE:T_PAD] += ych[-1, STRIDE:]
    div = np.zeros(T_PAD, np.float32)
    dv = div[:L * STRIDE].reshape(L, STRIDE)
    dv += 1.0
    dv[1:, :OVERLAP] += 1.0
    div[L * STRIDE:] += 1.0
    out /= (div + np.float32(1e-6))[None, :, None]
    # bias constants: v-bias passes through softmax-normalized attention
    # unchanged, so (b_v @ out_w.T + out_b) lands on every chunk row and
    # goes through the same divisor normalization
    const = (in_proj_b[512:].astype(np.float32) @ out_w.T.astype(np.float32)
             + out_b.astype(np.float32))
    out += const[None, None, :] * (div / (div + np.float32(1e-6)))[None, :, None]
    return out[:, :T, :].astype(dtype)


def kernel(x, in_proj_w, in_proj_b, out_w, out_b, _trace=False):
    x = np.asarray(x)
    in_proj_w = np.asarray(in_proj_w, np.float32)
    in_proj_b = np.asarray(in_proj_b, np.float32)
    out_w = np.asarray(out_w, np.float32)
    out_b = np.asarray(out_b, np.float32)

    nc = build_bass()
    in_maps = _shard_inputs(x, in_proj_w, in_proj_b)
    wto = np.ascontiguousarray(out_w.T.reshape(2, 128, 256)).astype(
        ml_dtypes.bfloat16)
    for m in in_maps:
        m["wto"] = wto

    res = run_bass_kernel_spmd(nc, in_maps, core_ids=list(range(8)),
                               trace=_trace)
    y_cores = [np.asarray(r["y"], np.float32) for r in res.results]
    out = _assemble(y_cores, in_proj_w, in_proj_b, out_w, out_b, x.dtype)
    kernel.last_results = res
    return out


# revision 13
# speedup vs baseline: 1.0113x; 1.0113x over previous
"""Chunked-attention Trainium2 kernel.

Problem (hardcoded shapes): x [4, 16384, 256] f32, in_proj_w [768, 256],
in_proj_b [768], out_w [256, 256], out_b [256].

Reference semantics: overlapping 128-token chunks (stride 96, overlap 32),
fused qkv projection, 8-head attention within each chunk, out projection,
overlap-add with divisor normalization.

Distribution: 8 cores = (batch b in 0..3) x (chunk-half in 0..1). Each core
processes 86 chunks of one batch (halves share chunk 85; the host drops the
duplicate). Each core receives its token window of x pre-transposed/cast to
bf16 ([d, t] layout) and emits raw per-chunk outputs [86, 128, 256] f32.
Overlap-add + divisor + bias constants are applied on the host (linear ops
that commute with the out projection).

Device pipeline per core:
  phase 1 (per 128-token tile): qkv projection for q,k in [e, tok] layout
    (weights stationary, xT moving), psum->sbuf copies with bias add + bf16
    cast.
  phase 2 (per chunk): v projection in [tok, e] layout; S^T = k_h q_h^T per
    head via row-packed tile_position matmuls (K=32); exp via ACT (scale
    folded); softmax denominators via ones-matmul (col-packed, broadcast to
    32 rows); O^T via col-packed matmuls (v_h stationary, P^T moving);
    normalization fused into the psum->sbuf copy; out projection; store.
"""

import numpy as np
import ml_dtypes
from contextlib import ExitStack

import concourse.bass as bass
import concourse.bacc as bacc
import concourse.mybir as mybir
import concourse.tile as tile
from concourse.bass_utils import run_bass_kernel_spmd


def _install_axon_ntff_hook():
    """Provide antenv.axon_hooks if the image lacks it, wired to the
    libaxon_pjrt.so NTFF profile ABI, so trace=True works under axon."""
    import sys, types, contextlib, ctypes
    try:
        from antenv.axon_hooks import get_axon_ntff_profile_hook  # noqa: F401
        return
    except ImportError:
        pass
    mod = types.ModuleType("antenv.axon_hooks")
    holder = [None]
    mod.set_axon_ntff_profile_hook = lambda h: holder.__setitem__(0, h)
    mod.get_axon_ntff_profile_hook = lambda: holder[0]
    sys.modules["antenv.axon_hooks"] = mod
    try:
        import antenv
        antenv.axon_hooks = mod
    except ImportError:
        pass
    so_path = "/opt/axon/libaxon_pjrt.so"
    try:
        lib = ctypes.CDLL(so_path)
        if not hasattr(lib, "axon_start_nrt_profile"):
            return
        lib.axon_start_nrt_profile.argtypes = [
            ctypes.POINTER(ctypes.c_int64), ctypes.c_size_t]
        lib.axon_start_nrt_profile.restype = ctypes.c_int64
        lib.axon_stop_nrt_profile.argtypes = [ctypes.c_char_p]
        lib.axon_stop_nrt_profile.restype = ctypes.c_int64

        @contextlib.contextmanager
        def _hook(output_dir, device_ids):
            import jax
            jax.devices()
            if device_ids:
                ids = (ctypes.c_int64 * len(device_ids))(*device_ids)
                rc = lib.axon_start_nrt_profile(ids, len(device_ids))
            else:
                rc = lib.axon_start_nrt_profile(None, 0)
            if rc != 0:
                raise RuntimeError(f"axon_start_nrt_profile rc={rc}")
            try:
                yield
            finally:
                n = lib.axon_stop_nrt_profile(str(output_dir).encode())
                print(f"profile: {n} file(s) written to {output_dir}")

        holder[0] = _hook
    except OSError:
        pass


_install_axon_ntff_hook()

F32 = mybir.dt.float32
BF16 = mybir.dt.bfloat16

DIM = 256
CHUNK = 128
OVERLAP = 32
STRIDE = 96
HEADS = 8
HD = 32
B = 4
T = 16384
L = 171              # total chunks per batch
T_PAD = L * STRIDE + OVERLAP  # 16448
SCALE = float(1.0 / np.sqrt(HD))

N_CHUNKS = 86        # chunks per core
N_TILES = 65         # 128-token qkv tiles per core
NTOK = N_TILES * 128  # 8320 padded local tokens (needs 86*96+32 = 8288)
T_HOST_PAD = STRIDE * (N_CHUNKS - 1) + NTOK  # 8160 + 8320 = 16480
SEG_TILES = 13       # xt DMA segment granularity (13 tiles = 1664 tokens)

_BASS_CACHE = {}


def build_bass(n_chunks=N_CHUNKS, n_tiles=N_TILES):
    key = (n_chunks, n_tiles)
    if key in _BASS_CACHE:
        return _BASS_CACHE[key]
    ntok = n_tiles * 128
    nc = bacc.Bacc(trn_type="TRN2", target_bir_lowering=False, debug=False)
    xt_d = nc.dram_tensor("xt", [2, 128, ntok], BF16, kind="ExternalInput")
    wt_d = nc.dram_tensor("wt", [2, 128, 768], BF16, kind="ExternalInput")
    wto_d = nc.dram_tensor("wto", [2, 128, 256], BF16, kind="ExternalInput")
    bqk_d = nc.dram_tensor("bqk", [128, 4], F32, kind="ExternalInput")
    y_d = nc.dram_tensor("y", [n_chunks, 128, 256], F32, kind="ExternalOutput")

    with tile.TileContext(nc) as tc, ExitStack() as ctx:
        _body(ctx, tc, xt_d[:], wt_d[:], wto_d[:], bqk_d[:], y_d[:],
              n_chunks, n_tiles)
    nc.compile()
    _BASS_CACHE[key] = nc
    return nc


def _body(ctx, tc, xt_d, wt_d, wto_d, bqk_d, y_d, n_chunks, n_tiles):
    nc = tc.nc
    ntok = n_tiles * 128

    consts = ctx.enter_context(tc.tile_pool(name="consts", bufs=1))
    big = ctx.enter_context(tc.tile_pool(name="big", bufs=1))
    sb = ctx.enter_context(tc.tile_pool(name="sb", bufs=3))
    sb2 = ctx.enter_context(tc.tile_pool(name="sb2", bufs=2))
    ps_big = ctx.enter_context(tc.tile_pool(name="ps_big", bufs=1, space="PSUM"))

    # Persistent PSUM layout (8 banks, 2KB/partition each). Concurrent
    # row-strip matmuls (tile_position row groups) must write distinct banks:
    # S^T strip h always lands in bank h. Col-strip matmuls (sums/O) may
    # share a bank (they write distinct partitions).
    #   b0-b3  stp: S^T strip h -> stp[:, h, 128g:128g+128]  (g = head group)
    #   b4-b5  sop: [chunk parity][g][ ot 0:128 | sums 128:256 ]
    #   b6     qkp: qkv projection tile psum
    #   b7     vyp: v psum [0:256] | y psum [256:512]
    stp = ps_big.tile([128, 4, 512], F32)
    sop = ps_big.tile([128, 2, 2, 256], F32)
    qkp = ps_big.tile([128, 4, 128], F32)
    vyp = ps_big.tile([128, 512], F32)

    # constants
    wt_sb = consts.tile([128, 2, 768], BF16)
    for j in range(2):
        nc.sync.dma_start(wt_sb[:, j, :], wt_d[j])
    wto_sb = consts.tile([128, 2, 256], BF16)
    for g in range(2):
        nc.sync.dma_start(wto_sb[:, g, :], wto_d[g])
    bqk_sb = consts.tile([128, 4], F32)
    nc.sync.dma_start(bqk_sb[:], bqk_d[:])
    ones_sb = consts.tile([128, 32], BF16)
    nc.vector.memset(ones_sb[:], 1.0)

    # span buffers
    xt_sb = big.tile([128, 2, ntok], BF16)
    qT_sb = big.tile([128, 2, ntok], BF16)
    kT_sb = big.tile([128, 2, ntok], BF16)

    n_segs = (n_tiles + SEG_TILES - 1) // SEG_TILES

    def load_seg(s):
        a = s * SEG_TILES * 128
        b = min(ntok, (s + 1) * SEG_TILES * 128)
        for j in range(2):
            nc.sync.dma_start(xt_sb[:, j, a:b], xt_d[j][:, a:b])

    def qkv_tile(t):
        a = t * 128
        for s in range(4):
            for j in range(2):
                nc.tensor.matmul(
                    qkp[:, s, :],
                    wt_sb[:, j, 128 * s:128 * (s + 1)],
                    xt_sb[:, j, a:a + 128],
                    start=(j == 0), stop=(j == 1),
                )
        # q slices on ACT (bias add + bf16 cast), k slices on DVE
        for s in range(2):
            nc.scalar.activation(
                qT_sb[:, s, a:a + 128], qkp[:, s, :],
                mybir.ActivationFunctionType.Identity,
                bias=bqk_sb[:, s:s + 1], scale=1.0,
            )
        for s in range(2, 4):
            nc.vector.tensor_scalar_add(
                kT_sb[:, s - 2, a:a + 128], qkp[:, s, :],
                bqk_sb[:, s:s + 1],
            )

    def attn_chunk(c):
        tok0 = c * STRIDE
        p = c % 2
        # S^T strips: head (g, h) -> bank h, cols 128g:128g+128
        for g in range(2):
            for h in range(4):
                p0 = 32 * h
                nc.tensor.matmul(
                    stp[:, h, 128 * g:128 * g + 128],
                    kT_sb[p0:p0 + 32, g, tok0:tok0 + 128],
                    qT_sb[p0:p0 + 32, g, tok0:tok0 + 128],
                    start=True, stop=True,
                    tile_position=(p0, 0),
                )
        # v projection: [tok, e] layout, xT stationary
        for j in range(2):
            nc.tensor.matmul(
                vyp[:, 0:256], xt_sb[:, j, tok0:tok0 + 128],
                wt_sb[:, j, 512:768],
                start=(j == 0), stop=(j == 1),
            )
        v_sb = sb.tile([128, 256], BF16, tag="v")
        nc.vector.tensor_copy(v_sb[:], vyp[:, 0:256])

        oc_sb = sb2.tile([128, 2, 128], BF16, tag="oc")
        for g in range(2):
            pt_sb = sb.tile([128, 4, 128], BF16, tag="pt")
            nc.scalar.activation(
                pt_sb[:], stp[:, :, 128 * g:128 * g + 128],
                mybir.ActivationFunctionType.Exp, scale=SCALE,
            )
            for h in range(4):
                p0 = 32 * h
                nc.tensor.matmul(
                    sop[p0:p0 + 32, p, g, 128:256],
                    ones_sb[:, :32], pt_sb[:, h, :],
                    start=True, stop=True,
                    tile_position=(0, p0),
                )
                nc.tensor.matmul(
                    sop[p0:p0 + 32, p, g, 0:128],
                    v_sb[:, 32 * (4 * g + h):32 * (4 * g + h) + 32],
                    pt_sb[:, h, :],
                    start=True, stop=True,
                    tile_position=(0, p0),
                )
        rb_sb = sb.tile([128, 2, 128], F32, tag="rb")
        nc.vector.reciprocal_approx_fast(rb_sb[:], sop[:, p, :, 128:256])
        nc.vector.tensor_mul(oc_sb[:], sop[:, p, :, 0:128], rb_sb[:])

        for g in range(2):
            nc.tensor.matmul(
                vyp[:, 256:512], oc_sb[:, g, :], wto_sb[:, g, :],
                start=(g == 0), stop=(g == 1),
            )
        y_sb = sb.tile([128, 256], F32, tag="y")
        nc.scalar.activation(y_sb[:, 0:128], vyp[:, 256:384],
                             mybir.ActivationFunctionType.Copy)
        nc.vector.tensor_copy(y_sb[:, 128:256], vyp[:, 384:512])
        nc.sync.dma_start(y_d[c], y_sb[:])

    # interleaved emission: load xt segments / project qkv tiles lazily,
    # then attention chunks as their token windows become available
    next_seg = 0
    next_tile = 0
    for c in range(n_chunks):
        t_hi = (c * STRIDE + 128 - 1) // 128  # last tile index chunk c needs
        while next_tile <= t_hi:
            while next_seg * SEG_TILES <= next_tile and next_seg < n_segs:
                load_seg(next_seg)
                next_seg += 1
            qkv_tile(next_tile)
            next_tile += 1
        attn_chunk(c)


def _shard_inputs(x, in_proj_w, in_proj_b):
    """Build per-core input maps. Core i = (batch i//2, half i%2)."""
    xp = np.zeros((B, T_HOST_PAD, DIM), np.float32)
    xp[:, :T, :] = x
    wt = np.ascontiguousarray(in_proj_w.T.reshape(2, 128, 768)).astype(
        ml_dtypes.bfloat16)
    bqk = np.ascontiguousarray(in_proj_b[:512].reshape(4, 128).T).astype(
        np.float32)
    in_maps = []
    for core in range(8):
        b, half = core // 2, core % 2
        t0 = half * STRIDE * (N_CHUNKS - 1)  # 0 or 8160
        seg = xp[b, t0:t0 + NTOK, :]
        xt = np.ascontiguousarray(seg.T.reshape(2, 128, NTOK)).astype(
            ml_dtypes.bfloat16)
        in_maps.append({"xt": xt, "wt": wt, "wto": None, "bqk": bqk})
    return in_maps


def _assemble(y_cores, in_proj_w, in_proj_b, out_w, out_b, dtype):
    """Host overlap-add + divisor + bias constants."""
    out = np.zeros((B, T_PAD, DIM), np.float32)
    for b in range(B):
        ych = np.empty((L, 128, DIM), np.float32)
        ych[:N_CHUNKS] = y_cores[2 * b]
        ych[N_CHUNKS:] = y_cores[2 * b + 1][1:]  # drop duplicated chunk 85
        ov = out[b, :L * STRIDE].reshape(L, STRIDE, DIM)
        ov += ych[:, :STRIDE]
        ov[1:, :OVERLAP] += ych[:-1, STRIDE:]
        out[b, L * STRIDE:T_PAD] += ych[-1, STRIDE:]
    div = np.zeros(T_PAD, np.float32)
    dv = div[:L * STRIDE].reshape(L, STRIDE)
    dv += 1.0
    dv[1:, :OVERLAP] += 1.0
    div[L * STRIDE:] += 1.0
    out /= (div + np.float32(1e-6))[None, :, None]
    # bias constants: v-bias passes through softmax-normalized attention
    # unchanged, so (b_v @ out_w.T + out_b) lands on every chunk row and
    # goes through the same divisor normalization
    const = (in_proj_b[512:].astype(np.float32) @ out_w.T.astype(np.float32)
             + out_b.astype(np.float32))
    out += const[None, None, :] * (div / (div + np.float32(1e-6)))[None, :, None]
    return out[:, :T, :].astype(dtype)


def kernel(x, in_proj_w, in_proj_b, out_w, out_b, _trace=False):
    x = np.asarray(x)
    in_proj_w = np.asarray(in_proj_w, np.float32)
    in_proj_b = np.asarray(in_proj_b, np.float32)
    out_w = np.asarray(out_w, np.float32)
    out_b = np.asarray(out_b, np.float32)

    nc = build_bass()
    in_maps = _shard_inputs(x, in_proj_w, in_proj_b)
    wto = np.ascontiguousarray(out_w.T.reshape(2, 128, 256)).astype(
        ml_dtypes.bfloat16)
    for m in in_maps:
        m["wto"] = wto

    res = run_bass_kernel_spmd(nc, in_maps, core_ids=list(range(8)),
                               trace=_trace)
    y_cores = [np.asarray(r["y"], np.float32) for r in res.results]
    out = _assemble(y_cores, in_proj_w, in_proj_b, out_w, out_b, x.dtype)
    kernel.last_results = res
    return out


# revision 17
# speedup vs baseline: 1.2850x; 1.2707x over previous
"""Chunked-attention Trainium2 kernel.

Problem (hardcoded shapes): x [4, 16384, 256] f32, in_proj_w [768, 256],
in_proj_b [768], out_w [256, 256], out_b [256].

Reference semantics: overlapping 128-token chunks (stride 96, overlap 32),
fused qkv projection, 8-head attention within each chunk, out projection,
overlap-add with divisor normalization.

Distribution: 8 cores = (batch b in 0..3) x (chunk-half in 0..1). Each core
processes 86 chunks of one batch (halves share chunk 85; the host drops the
duplicate). Each core receives its token window of x pre-transposed/cast to
bf16 ([d, t] layout) and emits raw per-chunk outputs [86, 128, 256] f32.
Overlap-add + divisor + bias constants are applied on the host (linear ops
that commute with the out projection).

Device pipeline per core:
  phase 1 (per 128-token tile): qkv projection for q,k in [e, tok] layout
    (weights stationary, xT moving), psum->sbuf copies with bias add + bf16
    cast.
  phase 2 (per chunk): v projection in [tok, e] layout; S^T = k_h q_h^T per
    head via row-packed tile_position matmuls (K=32); exp via ACT (scale
    folded); softmax denominators via ones-matmul (col-packed, broadcast to
    32 rows); O^T via col-packed matmuls (v_h stationary, P^T moving);
    normalization fused into the psum->sbuf copy; out projection; store.
"""

import numpy as np
import ml_dtypes
from contextlib import ExitStack

import concourse.bass as bass
import concourse.bacc as bacc
import concourse.mybir as mybir
import concourse.tile as tile
from concourse.bass_utils import run_bass_kernel_spmd


def _install_axon_ntff_hook():
    """Provide antenv.axon_hooks if the image lacks it, wired to the
    libaxon_pjrt.so NTFF profile ABI, so trace=True works under axon."""
    import sys, types, contextlib, ctypes
    try:
        from antenv.axon_hooks import get_axon_ntff_profile_hook  # noqa: F401
        return
    except ImportError:
        pass
    mod = types.ModuleType("antenv.axon_hooks")
    holder = [None]
    mod.set_axon_ntff_profile_hook = lambda h: holder.__setitem__(0, h)
    mod.get_axon_ntff_profile_hook = lambda: holder[0]
    sys.modules["antenv.axon_hooks"] = mod
    try:
        import antenv
        antenv.axon_hooks = mod
    except ImportError:
        pass
    so_path = "/opt/axon/libaxon_pjrt.so"
    try:
        lib = ctypes.CDLL(so_path)
        if not hasattr(lib, "axon_start_nrt_profile"):
            return
        lib.axon_start_nrt_profile.argtypes = [
            ctypes.POINTER(ctypes.c_int64), ctypes.c_size_t]
        lib.axon_start_nrt_profile.restype = ctypes.c_int64
        lib.axon_stop_nrt_profile.argtypes = [ctypes.c_char_p]
        lib.axon_stop_nrt_profile.restype = ctypes.c_int64

        @contextlib.contextmanager
        def _hook(output_dir, device_ids):
            import jax
            jax.devices()
            if device_ids:
                ids = (ctypes.c_int64 * len(device_ids))(*device_ids)
                rc = lib.axon_start_nrt_profile(ids, len(device_ids))
            else:
                rc = lib.axon_start_nrt_profile(None, 0)
            if rc != 0:
                raise RuntimeError(f"axon_start_nrt_profile rc={rc}")
            try:
                yield
            finally:
                n = lib.axon_stop_nrt_profile(str(output_dir).encode())
                print(f"profile: {n} file(s) written to {output_dir}")

        holder[0] = _hook
    except OSError:
        pass


_install_axon_ntff_hook()

F32 = mybir.dt.float32
BF16 = mybir.dt.bfloat16

DIM = 256
CHUNK = 128
OVERLAP = 32
STRIDE = 96
HEADS = 8
HD = 32
B = 4
T = 16384
L = 171              # total chunks per batch
T_PAD = L * STRIDE + OVERLAP  # 16448
SCALE = float(1.0 / np.sqrt(HD))

N_CHUNKS = 86        # chunks per core
N_TILES = 65         # 128-token qkv tiles per core
NTOK = N_TILES * 128  # 8320 padded local tokens (needs 86*96+32 = 8288)
T_HOST_PAD = STRIDE * (N_CHUNKS - 1) + NTOK  # 8160 + 8320 = 16480
SEG_TILES = 13       # xt DMA segment granularity (13 tiles = 1664 tokens)

_BASS_CACHE = {}


def build_bass(n_chunks=N_CHUNKS, n_tiles=N_TILES):
    key = (n_chunks, n_tiles)
    if key in _BASS_CACHE:
        return _BASS_CACHE[key]
    ntok = n_tiles * 128
    nc = bacc.Bacc(trn_type="TRN2", target_bir_lowering=False, debug=False)
    xt_d = nc.dram_tensor("xt", [2, 128, ntok], BF16, kind="ExternalInput")
    wt_d = nc.dram_tensor("wt", [2, 128, 768], BF16, kind="ExternalInput")
    wto_d = nc.dram_tensor("wto", [2, 128, 256], BF16, kind="ExternalInput")
    bqk_d = nc.dram_tensor("bqk", [128, 4], F32, kind="ExternalInput")
    y_d = nc.dram_tensor("y", [n_chunks, 128, 256], F32, kind="ExternalOutput")

    with tile.TileContext(nc) as tc, ExitStack() as ctx:
        _body(ctx, tc, xt_d[:], wt_d[:], wto_d[:], bqk_d[:], y_d[:],
              n_chunks, n_tiles)
    nc.compile()
    _BASS_CACHE[key] = nc
    return nc


def _body(ctx, tc, xt_d, wt_d, wto_d, bqk_d, y_d, n_chunks, n_tiles):
    nc = tc.nc
    ntok = n_tiles * 128

    consts = ctx.enter_context(tc.tile_pool(name="consts", bufs=1))
    big = ctx.enter_context(tc.tile_pool(name="big", bufs=1))
    sb = ctx.enter_context(tc.tile_pool(name="sb", bufs=3))
    sb2 = ctx.enter_context(tc.tile_pool(name="sb2", bufs=2))
    ps_big = ctx.enter_context(tc.tile_pool(name="ps_big", bufs=1, space="PSUM"))

    # Chunks are processed in pairs (supersteps). Persistent PSUM layout
    # (8 banks x 2KB/partition). Concurrent row-strip matmuls
    # (tile_position row groups) must land in distinct banks: S^T strip h
    # always writes bank h. Col-strip matmuls may share a bank (they write
    # distinct partitions).
    #   b0-b3  stp: S^T strip (cb, g, h) -> stp[:, h, 128*(2cb+g) : +128]
    #   b4-b5  sop: [cb][g][ ot 0:128 | sums 128:256 ]
    #   b6     vp:  v psum per chunk-in-pair
    #   b7     yp:  y psum per chunk-in-pair
    # The qkv projection phase runs BEFORE any attention work and reuses
    # banks via its own double-buffered pool.
    stp = ps_big.tile([128, 4, 512], F32)
    sop = ps_big.tile([128, 2, 2, 256], F32)
    vp = ps_big.tile([128, 2, 256], F32)
    yp = ps_big.tile([128, 2, 256], F32)

    # constants
    wt_sb = consts.tile([128, 2, 768], BF16)
    for j in range(2):
        nc.sync.dma_start(wt_sb[:, j, :], wt_d[j])
    wto_sb = consts.tile([128, 2, 256], BF16)
    for g in range(2):
        nc.sync.dma_start(wto_sb[:, g, :], wto_d[g])
    bqk_sb = consts.tile([128, 4], F32)
    nc.sync.dma_start(bqk_sb[:], bqk_d[:])
    ones_sb = consts.tile([128, 32], BF16)
    nc.vector.memset(ones_sb[:], 1.0)

    # span buffers
    xt_sb = big.tile([128, 2, ntok], BF16)
    qT_sb = big.tile([128, 2, ntok], BF16)
    kT_sb = big.tile([128, 2, ntok], BF16)

    n_segs = (n_tiles + SEG_TILES - 1) // SEG_TILES

    def load_seg(s):
        a = s * SEG_TILES * 128
        b = min(ntok, (s + 1) * SEG_TILES * 128)
        for j in range(2):
            nc.sync.dma_start(xt_sb[:, j, a:b], xt_d[j][:, a:b])

    def qkv_pair(t2):
        # two 128-token tiles per psum round; stp cols ping-pong by parity
        a = t2 * 256
        w = min(256, ntok - a)
        base = 256 * (t2 % 2)
        for s in range(4):
            for j in range(2):
                nc.tensor.matmul(
                    stp[:, s, base:base + w],
                    wt_sb[:, j, 128 * s:128 * (s + 1)],
                    xt_sb[:, j, a:a + w],
                    start=(j == 0), stop=(j == 1),
                )
        # q slices on ACT (bias add + bf16 cast), k slices on DVE
        for s in range(2):
            nc.scalar.activation(
                qT_sb[:, s, a:a + w], stp[:, s, base:base + w],
                mybir.ActivationFunctionType.Identity,
                bias=bqk_sb[:, s:s + 1], scale=1.0,
            )
        for s in range(2, 4):
            nc.vector.tensor_scalar_add(
                kT_sb[:, s - 2, a:a + w], stp[:, s, base:base + w],
                bqk_sb[:, s:s + 1],
            )

    def attn_pair(pr):
        # chunks 2*pr and 2*pr+1 in one superstep
        # S^T strips: (cb, g, h) -> bank h, cols 128*(2cb+g)
        for cb in range(2):
            tok0 = (2 * pr + cb) * STRIDE
            for g in range(2):
                for h in range(4):
                    p0 = 32 * h
                    nc.tensor.matmul(
                        stp[:, h, 128 * (2 * cb + g):128 * (2 * cb + g) + 128],
                        kT_sb[p0:p0 + 32, g, tok0:tok0 + 128],
                        qT_sb[p0:p0 + 32, g, tok0:tok0 + 128],
                        start=True, stop=True,
                        tile_position=(p0, 0),
                    )
        # v projections
        for cb in range(2):
            tok0 = (2 * pr + cb) * STRIDE
            for j in range(2):
                nc.tensor.matmul(
                    vp[:, cb, :], xt_sb[:, j, tok0:tok0 + 128],
                    wt_sb[:, j, 512:768],
                    start=(j == 0), stop=(j == 1),
                )
        # exp over the whole pair in one ACT op
        pt_sb = sb.tile([128, 4, 512], BF16, tag="pt")
        nc.scalar.activation(
            pt_sb[:], stp[:],
            mybir.ActivationFunctionType.Exp, scale=SCALE,
        )
        v_sb = sb.tile([128, 2, 256], BF16, tag="v")
        nc.vector.tensor_copy(v_sb[:], vp[:])
        # sums + O col-strips
        for cb in range(2):
            for g in range(2):
                for h in range(4):
                    p0 = 32 * h
                    col = 128 * (2 * cb + g)
                    nc.tensor.matmul(
                        sop[p0:p0 + 32, cb, g, 128:256],
                        ones_sb[:, :32], pt_sb[:, h, col:col + 128],
                        start=True, stop=True,
                        tile_position=(0, p0),
                    )
                    nc.tensor.matmul(
                        sop[p0:p0 + 32, cb, g, 0:128],
                        v_sb[:, cb, 32 * (4 * g + h):32 * (4 * g + h) + 32],
                        pt_sb[:, h, col:col + 128],
                        start=True, stop=True,
                        tile_position=(0, p0),
                    )
        rb_sb = sb.tile([128, 2, 2, 128], F32, tag="rb")
        nc.vector.reciprocal_approx_fast(
            rb_sb[:].rearrange("p a b c -> p (a b) c"),
            sop[:, :, :, 128:256].rearrange("p a b c -> p (a b) c"))
        oc_sb = sb2.tile([128, 2, 2, 128], BF16, tag="oc")
        nc.vector.tensor_mul(oc_sb[:], sop[:, :, :, 0:128], rb_sb[:])
        # out projection
        for cb in range(2):
            for g in range(2):
                nc.tensor.matmul(
                    yp[:, cb, :], oc_sb[:, cb, g, :], wto_sb[:, g, :],
                    start=(g == 0), stop=(g == 1),
                )
        y_sb = sb.tile([128, 2, 256], F32, tag="y")
        nc.scalar.activation(y_sb[:, :, 0:128], yp[:, :, 0:128],
                             mybir.ActivationFunctionType.Copy)
        nc.vector.tensor_copy(y_sb[:, :, 128:256], yp[:, :, 128:256])
        y_view = y_d[2 * pr:2 * pr + 2].rearrange("c p d -> p c d")
        nc.sync.dma_start(y_view, y_sb[:])

    # phase 1: stream all xt segments + qkv projection tile-pairs
    for s in range(n_segs):
        load_seg(s)
    for t2 in range((n_tiles + 1) // 2):
        qkv_pair(t2)
    # phase 2: attention chunk-pairs
    assert n_chunks % 2 == 0
    for pr in range(n_chunks // 2):
        attn_pair(pr)


def _shard_inputs(x, in_proj_w, in_proj_b):
    """Build per-core input maps. Core i = (batch i//2, half i%2)."""
    xp = np.zeros((B, T_HOST_PAD, DIM), np.float32)
    xp[:, :T, :] = x
    wt = np.ascontiguousarray(in_proj_w.T.reshape(2, 128, 768)).astype(
        ml_dtypes.bfloat16)
    bqk = np.ascontiguousarray(in_proj_b[:512].reshape(4, 128).T).astype(
        np.float32)
    in_maps = []
    for core in range(8):
        b, half = core // 2, core % 2
        t0 = half * STRIDE * (N_CHUNKS - 1)  # 0 or 8160
        seg = xp[b, t0:t0 + NTOK, :]
        xt = np.ascontiguousarray(seg.T.reshape(2, 128, NTOK)).astype(
            ml_dtypes.bfloat16)
        in_maps.append({"xt": xt, "wt": wt, "wto": None, "bqk": bqk})
    return in_maps


def _assemble(y_cores, in_proj_w, in_proj_b, out_w, out_b, dtype):
    """Host overlap-add + divisor + bias constants."""
    out = np.zeros((B, T_PAD, DIM), np.float32)
    for b in range(B):
        ych = np.empty((L, 128, DIM), np.float32)
        ych[:N_CHUNKS] = y_cores[2 * b]
        ych[N_CHUNKS:] = y_cores[2 * b + 1][1:]  # drop duplicated chunk 85
        ov = out[b, :L * STRIDE].reshape(L, STRIDE, DIM)
        ov += ych[:, :STRIDE]
        ov[1:, :OVERLAP] += ych[:-1, STRIDE:]
        out[b, L * STRIDE:T_PAD] += ych[-1, STRIDE:]
    div = np.zeros(T_PAD, np.float32)
    dv = div[:L * STRIDE].reshape(L, STRIDE)
    dv += 1.0
    dv[1:, :OVERLAP] += 1.0
    div[L * STRIDE:] += 1.0
    out /= (div + np.float32(1e-6))[None, :, None]
    # bias constants: v-bias passes through softmax-normalized attention
    # unchanged, so (b_v @ out_w.T + out_b) lands on every chunk row and
    # goes through the same divisor normalization
    const = (in_proj_b[512:].astype(np.float32) @ out_w.T.astype(np.float32)
             + out_b.astype(np.float32))
    out += const[None, None, :] * (div / (div + np.float32(1e-6)))[None, :, None]
    return out[:, :T, :].astype(dtype)


def kernel(x, in_proj_w, in_proj_b, out_w, out_b, _trace=False):
    x = np.asarray(x)
    in_proj_w = np.asarray(in_proj_w, np.float32)
    in_proj_b = np.asarray(in_proj_b, np.float32)
    out_w = np.asarray(out_w, np.float32)
    out_b = np.asarray(out_b, np.float32)

    nc = build_bass()
    in_maps = _shard_inputs(x, in_proj_w, in_proj_b)
    wto = np.ascontiguousarray(out_w.T.reshape(2, 128, 256)).astype(
        ml_dtypes.bfloat16)
    for m in in_maps:
        m["wto"] = wto

    res = run_bass_kernel_spmd(nc, in_maps, core_ids=list(range(8)),
                               trace=_trace)
    y_cores = [np.asarray(r["y"], np.float32) for r in res.results]
    out = _assemble(y_cores, in_proj_w, in_proj_b, out_w, out_b, x.dtype)
    kernel.last_results = res
    return out


# revision 20
# speedup vs baseline: 1.3340x; 1.0382x over previous
"""Chunked-attention Trainium2 kernel.

Problem (hardcoded shapes): x [4, 16384, 256] f32, in_proj_w [768, 256],
in_proj_b [768], out_w [256, 256], out_b [256].

Reference semantics: overlapping 128-token chunks (stride 96, overlap 32),
fused qkv projection, 8-head attention within each chunk, out projection,
overlap-add with divisor normalization.

Distribution: 8 cores = (batch b in 0..3) x (chunk-half in 0..1). Each core
processes 86 chunks of one batch (halves share chunk 85; the host drops the
duplicate). Each core receives its token window of x pre-transposed/cast to
bf16 ([d, t] layout) and emits raw per-chunk outputs [86, 128, 256] f32.
Overlap-add + divisor + bias constants are applied on the host (linear ops
that commute with the out projection).

Device pipeline per core:
  phase 1 (per 128-token tile): qkv projection for q,k in [e, tok] layout
    (weights stationary, xT moving), psum->sbuf copies with bias add + bf16
    cast.
  phase 2 (per chunk): v projection in [tok, e] layout; S^T = k_h q_h^T per
    head via row-packed tile_position matmuls (K=32); exp via ACT (scale
    folded); softmax denominators via ones-matmul (col-packed, broadcast to
    32 rows); O^T via col-packed matmuls (v_h stationary, P^T moving);
    normalization fused into the psum->sbuf copy; out projection; store.
"""

import numpy as np
import ml_dtypes
from contextlib import ExitStack

import concourse.bass as bass
import concourse.bacc as bacc
import concourse.mybir as mybir
import concourse.tile as tile
from concourse.bass_utils import run_bass_kernel_spmd


def _install_axon_ntff_hook():
    """Provide antenv.axon_hooks if the image lacks it, wired to the
    libaxon_pjrt.so NTFF profile ABI, so trace=True works under axon."""
    import sys, types, contextlib, ctypes
    try:
        from antenv.axon_hooks import get_axon_ntff_profile_hook  # noqa: F401
        return
    except ImportError:
        pass
    mod = types.ModuleType("antenv.axon_hooks")
    holder = [None]
    mod.set_axon_ntff_profile_hook = lambda h: holder.__setitem__(0, h)
    mod.get_axon_ntff_profile_hook = lambda: holder[0]
    sys.modules["antenv.axon_hooks"] = mod
    try:
        import antenv
        antenv.axon_hooks = mod
    except ImportError:
        pass
    so_path = "/opt/axon/libaxon_pjrt.so"
    try:
        lib = ctypes.CDLL(so_path)
        if not hasattr(lib, "axon_start_nrt_profile"):
            return
        lib.axon_start_nrt_profile.argtypes = [
            ctypes.POINTER(ctypes.c_int64), ctypes.c_size_t]
        lib.axon_start_nrt_profile.restype = ctypes.c_int64
        lib.axon_stop_nrt_profile.argtypes = [ctypes.c_char_p]
        lib.axon_stop_nrt_profile.restype = ctypes.c_int64

        @contextlib.contextmanager
        def _hook(output_dir, device_ids):
            import jax
            jax.devices()
            if device_ids:
                ids = (ctypes.c_int64 * len(device_ids))(*device_ids)
                rc = lib.axon_start_nrt_profile(ids, len(device_ids))
            else:
                rc = lib.axon_start_nrt_profile(None, 0)
            if rc != 0:
                raise RuntimeError(f"axon_start_nrt_profile rc={rc}")
            try:
                yield
            finally:
                n = lib.axon_stop_nrt_profile(str(output_dir).encode())
                print(f"profile: {n} file(s) written to {output_dir}")

        holder[0] = _hook
    except OSError:
        pass


_install_axon_ntff_hook()

F32 = mybir.dt.float32
BF16 = mybir.dt.bfloat16

DIM = 256
CHUNK = 128
OVERLAP = 32
STRIDE = 96
HEADS = 8
HD = 32
B = 4
T = 16384
L = 171              # total chunks per batch
T_PAD = L * STRIDE + OVERLAP  # 16448
SCALE = float(1.0 / np.sqrt(HD))

N_CHUNKS = 86        # chunks per core
N_TILES = 65         # 128-token qkv tiles per core
NTOK = N_TILES * 128  # 8320 padded local tokens (needs 86*96+32 = 8288)
T_HOST_PAD = STRIDE * (N_CHUNKS - 1) + NTOK  # 8160 + 8320 = 16480
SEG_TILES = 13       # xt DMA segment granularity (13 tiles = 1664 tokens)

_BASS_CACHE = {}


def build_bass(n_chunks=N_CHUNKS, n_tiles=N_TILES):
    key = (n_chunks, n_tiles)
    if key in _BASS_CACHE:
        return _BASS_CACHE[key]
    ntok = n_tiles * 128
    nc = bacc.Bacc(trn_type="TRN2", target_bir_lowering=False, debug=False)
    xt_d = nc.dram_tensor("xt", [2, 128, ntok], BF16, kind="ExternalInput")
    wt_d = nc.dram_tensor("wt", [2, 128, 768], BF16, kind="ExternalInput")
    wto_d = nc.dram_tensor("wto", [2, 128, 256], BF16, kind="ExternalInput")
    bqk_d = nc.dram_tensor("bqk", [128, 4], F32, kind="ExternalInput")
    y_d = nc.dram_tensor("y", [n_chunks, 128, 256], F32, kind="ExternalOutput")

    with tile.TileContext(nc) as tc, ExitStack() as ctx:
        _body(ctx, tc, xt_d[:], wt_d[:], wto_d[:], bqk_d[:], y_d[:],
              n_chunks, n_tiles)
    nc.compile()
    _BASS_CACHE[key] = nc
    return nc


def _body(ctx, tc, xt_d, wt_d, wto_d, bqk_d, y_d, n_chunks, n_tiles):
    nc = tc.nc
    ntok = n_tiles * 128

    consts = ctx.enter_context(tc.tile_pool(name="consts", bufs=1))
    big = ctx.enter_context(tc.tile_pool(name="big", bufs=1))
    sb = ctx.enter_context(tc.tile_pool(name="sb", bufs=3))
    sb2 = ctx.enter_context(tc.tile_pool(name="sb2", bufs=2))
    ps_big = ctx.enter_context(tc.tile_pool(name="ps_big", bufs=1, space="PSUM"))

    # Chunks are processed in pairs (supersteps). Persistent PSUM layout
    # (8 banks x 2KB/partition). Two hardware constraints drive it:
    # concurrent row-strip matmuls (tile_position row groups) must land in
    # distinct banks, and a PE-write concurrent with another engine's read
    # of the SAME bank is serialized by the tile tracker (bank granularity)
    # - so double-buffering must alternate banks, not columns.
    #   b0-b3  stp: S^T strip (cb, g, h) -> stp[:, h, 128*(2cb+g) : +128]
    #          (phase 1 reuses these: even tile-pair -> banks 0-1,
    #           odd tile-pair -> banks 2-3)
    #   b4     sums: strip h -> sop[32h:32h+32, 0, :, :]
    #   b5     ot:   (cb,g,h) -> sop[32h:32h+32, 1, 2cb+g, :]
    #   b6     vp:  v psum per chunk-in-pair
    #   b7     yp:  y psum per chunk-in-pair
    stp = ps_big.tile([128, 4, 512], F32)
    sop = ps_big.tile([128, 2, 4, 128], F32)
    vp = ps_big.tile([128, 2, 256], F32)
    yp = ps_big.tile([128, 2, 256], F32)

    # constants
    wt_sb = consts.tile([128, 2, 768], BF16)
    for j in range(2):
        nc.sync.dma_start(wt_sb[:, j, :], wt_d[j])
    wto_sb = consts.tile([128, 2, 256], BF16)
    for g in range(2):
        nc.sync.dma_start(wto_sb[:, g, :], wto_d[g])
    bqk_sb = consts.tile([128, 4], F32)
    nc.sync.dma_start(bqk_sb[:], bqk_d[:])
    ones_sb = consts.tile([128, 32], BF16)
    nc.vector.memset(ones_sb[:], 1.0)

    # span buffers
    xt_sb = big.tile([128, 2, ntok], BF16)
    qT_sb = big.tile([128, 2, ntok], BF16)
    kT_sb = big.tile([128, 2, ntok], BF16)

    n_segs = (n_tiles + SEG_TILES - 1) // SEG_TILES

    def load_seg(s):
        a = s * SEG_TILES * 128
        b = min(ntok, (s + 1) * SEG_TILES * 128)
        for j in range(2):
            nc.sync.dma_start(xt_sb[:, j, a:b], xt_d[j][:, a:b])

    def qkv_pair(t2):
        # two 128-token tiles per psum round; banks ping-pong by parity:
        # even pairs use stp banks 0-1, odd pairs banks 2-3 (slice s at
        # bank 2*par + s//2, cols 256*(s%2))
        a = t2 * 256
        w = min(256, ntok - a)
        bb = 2 * (t2 % 2)

        def qk_ps(s):
            return stp[:, bb + s // 2, 256 * (s % 2):256 * (s % 2) + w]

        for s in range(4):
            for j in range(2):
                nc.tensor.matmul(
                    qk_ps(s),
                    wt_sb[:, j, 128 * s:128 * (s + 1)],
                    xt_sb[:, j, a:a + w],
                    start=(j == 0), stop=(j == 1),
                )
        # q slices on ACT (bias add + bf16 cast), k slices on DVE
        for s in range(2):
            nc.scalar.activation(
                qT_sb[:, s, a:a + w], qk_ps(s),
                mybir.ActivationFunctionType.Identity,
                bias=bqk_sb[:, s:s + 1], scale=1.0,
            )
        for s in range(2, 4):
            nc.vector.tensor_scalar_add(
                kT_sb[:, s - 2, a:a + w], qk_ps(s),
                bqk_sb[:, s:s + 1],
            )

    def attn_pair_main(pr):
        # chunks 2*pr and 2*pr+1 in one superstep
        # S^T strips: (cb, g, h) -> bank h, cols 128*(2cb+g)
        for cb in range(2):
            tok0 = (2 * pr + cb) * STRIDE
            for g in range(2):
                for h in range(4):
                    p0 = 32 * h
                    nc.tensor.matmul(
                        stp[:, h, 128 * (2 * cb + g):128 * (2 * cb + g) + 128],
                        kT_sb[p0:p0 + 32, g, tok0:tok0 + 128],
                        qT_sb[p0:p0 + 32, g, tok0:tok0 + 128],
                        start=True, stop=True,
                        tile_position=(p0, 0),
                    )
        # v projections
        for cb in range(2):
            tok0 = (2 * pr + cb) * STRIDE
            for j in range(2):
                nc.tensor.matmul(
                    vp[:, cb, :], xt_sb[:, j, tok0:tok0 + 128],
                    wt_sb[:, j, 512:768],
                    start=(j == 0), stop=(j == 1),
                )
        # exp over the whole pair in one ACT op
        pt_sb = sb.tile([128, 4, 512], BF16, tag="pt")
        nc.scalar.activation(
            pt_sb[:], stp[:],
            mybir.ActivationFunctionType.Exp, scale=SCALE,
        )
        v_sb = sb.tile([128, 2, 256], BF16, tag="v")
        nc.vector.tensor_copy(v_sb[:], vp[:])
        # softmax denominators: 4 col-strips, each N=512 over all (cb,g)
        for h in range(4):
            p0 = 32 * h
            nc.tensor.matmul(
                sop[p0:p0 + 32, 0, :, :],
                ones_sb[:, :32], pt_sb[:, h, :],
                start=True, stop=True,
                tile_position=(0, p0),
            )
        # O^T col-strips
        for cb in range(2):
            for g in range(2):
                for h in range(4):
                    p0 = 32 * h
                    col = 128 * (2 * cb + g)
                    nc.tensor.matmul(
                        sop[p0:p0 + 32, 1, 2 * cb + g, :],
                        v_sb[:, cb, 128 * g + p0:128 * g + p0 + 32],
                        pt_sb[:, h, col:col + 128],
                        start=True, stop=True,
                        tile_position=(0, p0),
                    )
        rb_sb = sb.tile([128, 4, 128], F32, tag="rb")
        nc.vector.reciprocal_approx_fast(rb_sb[:], sop[:, 0, :, :])
        oc_sb = sb2.tile([128, 4, 128], BF16, tag="oc")
        nc.vector.tensor_mul(oc_sb[:], sop[:, 1, :, :], rb_sb[:])
        return oc_sb

    def attn_pair_tail(pr, oc_sb):
        # out projection + store, emitted one superstep later so the ACT/DVE
        # queues are never blocked behind an in-flight chain
        for cb in range(2):
            for g in range(2):
                nc.tensor.matmul(
                    yp[:, cb, :], oc_sb[:, 2 * cb + g, :], wto_sb[:, g, :],
                    start=(g == 0), stop=(g == 1),
                )
        y_sb = sb.tile([128, 2, 256], F32, tag="y")
        nc.scalar.activation(y_sb[:, :, 0:128], yp[:, :, 0:128],
                             mybir.ActivationFunctionType.Copy)
        nc.vector.tensor_copy(y_sb[:, :, 128:256], yp[:, :, 128:256])
        y_view = y_d[2 * pr:2 * pr + 2].rearrange("c p d -> p c d")
        nc.sync.dma_start(y_view, y_sb[:])

    # phase 1: stream all xt segments + qkv projection tile-pairs
    for s in range(n_segs):
        load_seg(s)
    for t2 in range((n_tiles + 1) // 2):
        qkv_pair(t2)
    # phase 2: attention chunk-pairs, out-projection stage-shifted by one
    assert n_chunks % 2 == 0
    prev = None
    for pr in range(n_chunks // 2):
        oc = attn_pair_main(pr)
        if prev is not None:
            attn_pair_tail(pr - 1, prev)
        prev = oc
    attn_pair_tail(n_chunks // 2 - 1, prev)


def _shard_inputs(x, in_proj_w, in_proj_b):
    """Build per-core input maps. Core i = (batch i//2, half i%2)."""
    xp = np.zeros((B, T_HOST_PAD, DIM), np.float32)
    xp[:, :T, :] = x
    wt = np.ascontiguousarray(in_proj_w.T.reshape(2, 128, 768)).astype(
        ml_dtypes.bfloat16)
    bqk = np.ascontiguousarray(in_proj_b[:512].reshape(4, 128).T).astype(
        np.float32)
    in_maps = []
    for core in range(8):
        b, half = core // 2, core % 2
        t0 = half * STRIDE * (N_CHUNKS - 1)  # 0 or 8160
        seg = xp[b, t0:t0 + NTOK, :]
        xt = np.ascontiguousarray(seg.T.reshape(2, 128, NTOK)).astype(
            ml_dtypes.bfloat16)
        in_maps.append({"xt": xt, "wt": wt, "wto": None, "bqk": bqk})
    return in_maps


def _assemble(y_cores, in_proj_w, in_proj_b, out_w, out_b, dtype):
    """Host overlap-add + divisor + bias constants."""
    out = np.zeros((B, T_PAD, DIM), np.float32)
    for b in range(B):
        ych = np.empty((L, 128, DIM), np.float32)
        ych[:N_CHUNKS] = y_cores[2 * b]
        ych[N_CHUNKS:] = y_cores[2 * b + 1][1:]  # drop duplicated chunk 85
        ov = out[b, :L * STRIDE].reshape(L, STRIDE, DIM)
        ov += ych[:, :STRIDE]
        ov[1:, :OVERLAP] += ych[:-1, STRIDE:]
        out[b, L * STRIDE:T_PAD] += ych[-1, STRIDE:]
    div = np.zeros(T_PAD, np.float32)
    dv = div[:L * STRIDE].reshape(L, STRIDE)
    dv += 1.0
    dv[1:, :OVERLAP] += 1.0
    div[L * STRIDE:] += 1.0
    out /= (div + np.float32(1e-6))[None, :, None]
    # bias constants: v-bias passes through softmax-normalized attention
    # unchanged, so (b_v @ out_w.T + out_b) lands on every chunk row and
    # goes through the same divisor normalization
    const = (in_proj_b[512:].astype(np.float32) @ out_w.T.astype(np.float32)
             + out_b.astype(np.float32))
    out += const[None, None, :] * (div / (div + np.float32(1e-6)))[None, :, None]
    return out[:, :T, :].astype(dtype)


def kernel(x, in_proj_w, in_proj_b, out_w, out_b, _trace=False):
    x = np.asarray(x)
    in_proj_w = np.asarray(in_proj_w, np.float32)
    in_proj_b = np.asarray(in_proj_b, np.float32)
    out_w = np.asarray(out_w, np.float32)
    out_b = np.asarray(out_b, np.float32)

    nc = build_bass()
    in_maps = _shard_inputs(x, in_proj_w, in_proj_b)
    wto = np.ascontiguousarray(out_w.T.reshape(2, 128, 256)).astype(
        ml_dtypes.bfloat16)
    for m in in_maps:
        m["wto"] = wto

    res = run_bass_kernel_spmd(nc, in_maps, core_ids=list(range(8)),
                               trace=_trace)
    y_cores = [np.asarray(r["y"], np.float32) for r in res.results]
    out = _assemble(y_cores, in_proj_w, in_proj_b, out_w, out_b, x.dtype)
    kernel.last_results = res
    return out
